# revision 7
# baseline (speedup 1.0000x reference)
"""MLA decode paged attention (flat_pa_mla latent-cache path) on 8 TRN2 NeuronCores.

Sharding: data-parallel over the batch axis — each core owns 4 complete requests
and computes its slice of the output independently, no collectives.

Two key optimizations over the dense baseline:

1. block_bias masks the unused tail of every paged block (avg usage 64/128) and
   masked positions contribute exactly zero, so host prep gathers ONLY the used
   positions of each request's 16 blocks, packed into T = ceil(max_used/128)
   tiles of 128 (padded with zero-K/zero-V and bias=-1e9): ~55% of dense bytes.
   The kernel is DMA-bound, so HW time tracks bytes almost linearly.

2. No max-subtraction: logits are O(6) for this distribution, so exp() is safe
   in f32 and the softmax is computed as p = exp(qk + bias), o = (sum p v) /
   (sum p). This removes the all-tiles max/rescale barrier between QK and PV —
   PV accumulation pipelines tile-by-tile inside the DMA stream instead of
   running serially after it.

DMA layout: HWDGE descriptor generation (~70ns/descriptor) caps a ring at
~size_of(per-partition run)/70ns, so blobs are merged for fat contiguous
per-partition runs:
  ktl [2, 128, 2, 4, CAP]: K^T lora rows per request-PAIR (20KB/partition).
  ktr [65, 4, CAP]: rope rows + bias row, all requests, one early DMA.
  vh  [128, T, 4, 512]: natural-layout V tiles, 3 chunk DMAs (16/16/8KB runs).
  qt  [4, 577, 16]: per-request SCALE*query transposed, trailing 1.0 row.

Device (per core), 4 requests in lockstep at 32-partition stride (PE column
groups via tile_position):
  pass A per position-group (<=4 tiles): per request 5 PE matmuls accumulate
  qk+bias into a PSUM bank; ACT exp -> p tiles (bf16), DVE per-group sums.
  pass B per tile: PE-transpose p, 4 PV matmuls accumulate [128,512] output.
  Finalize: o = po * (1/sum p) broadcast, one DMA out.
"""

import numpy as np

import concourse.bass as bass
import concourse.mybir as mybir
import concourse.tile as tile
from concourse import bacc
from concourse.bass_utils import run_bass_kernel_spmd
from concourse.masks import make_identity

B = 32
H = 16
KVL = 512
ROPE = 64
D = KVL + ROPE          # 576
BS = 128
BPS = 16                # blocks per request (input format)
NB = B * BPS            # 512
SCALE = 192 ** -0.5
NEG = -1.0e9
NCORES = 8
RPC = B // NCORES       # 4 requests per core
DR = D + 1              # 577 rows: 576 latent+rope dims + 1 bias row
RR = DR - 512           # 65 rope+bias rows
RST = 32                # per-request partition stride (PE col groups are 32-wide)
HP = RPC * RST          # 128 partitions spanned by packed per-request ops

KV_DT = mybir.dt.bfloat16
P_DT = mybir.dt.bfloat16

TRACE = False           # set True (with profhook installed) to NTFF-profile
LAST_RESULTS = None     # BassKernelResults of the last kernel() call when TRACE

_NC_CACHE = {}


def _np_of(dt):
    import ml_dtypes

    return {mybir.dt.float32: np.float32, mybir.dt.bfloat16: ml_dtypes.bfloat16}[dt]


def _groups(T):
    """Split T position tiles into QK groups of <=4 tiles (N<=512)."""
    gt = []
    t = T
    while t > 0:
        g = min(4, t)
        gt.append(g)
        t -= g
    offs = np.cumsum([0] + gt).tolist()
    return gt, offs


def _vchunks(T):
    """Split T position tiles into <=3 vh DMA chunks.

    Chunk 0 (first half) rides the gpsimd SWDGE ring so PV can start as soon
    as pass A produces p tiles; chunks 1/2 ride sync/scalar after the K blobs,
    with the last chunk smallest to shrink the post-DMA tail.
    """
    if T <= 4:
        return [(0, T)]
    a = (T + 1) // 2
    b = T - 2 if T - 2 > a else T
    cuts = [0, a, b, T]
    return [(cuts[i], cuts[i + 1]) for i in range(3) if cuts[i] < cuts[i + 1]]


def _build(T, kv_dt, p_dt):
    f32 = mybir.dt.float32
    CAP = T * BS
    GT, OFFS = _groups(T)
    VC = _vchunks(T)
    nc = bacc.Bacc("TRN2", target_bir_lowering=False, debug=False)
    ktl = nc.dram_tensor(
        "ktl", [2, 128, 2, 4, CAP], kv_dt, kind="ExternalInput"
    ).ap()
    ktr = nc.dram_tensor("ktr", [RR, RPC, CAP], kv_dt, kind="ExternalInput").ap()
    vh = nc.dram_tensor("vh", [BS, T, RPC, KVL], kv_dt, kind="ExternalInput").ap()
    qt = nc.dram_tensor("qt", [RPC, DR, H], kv_dt, kind="ExternalInput").ap()
    o = nc.dram_tensor("o", [RPC, H, KVL], f32, kind="ExternalOutput").ap()

    with tile.TileContext(nc) as tc:
        with (
            tc.tile_pool(name="singles", bufs=1) as singles,
            tc.tile_pool(name="pp", bufs=4) as pp,
            tc.tile_pool(name="stats", bufs=8) as stats,
            tc.tile_pool(name="pap", bufs=2, space="PSUM") as pap,
            tc.tile_pool(name="ptpp", bufs=2, space="PSUM") as ptpp,
            tc.tile_pool(name="pop", bufs=1, space="PSUM") as pop,
        ):
            # qt first: it's the lhsT of every pass-A matmul.
            qt1 = singles.tile([128, RPC, 4, H], kv_dt)
            qt2 = singles.tile([RR, RPC, H], kv_dt)
            for r in range(RPC):
                nc.gpsimd.dma_start(
                    out=qt1[:, r, :, :],
                    in_=qt[r, 0 : 4 * 128, :].rearrange("(c p) h -> p c h", p=128),
                )
                nc.gpsimd.dma_start(out=qt2[:, r, :], in_=qt[r, 512:DR, :])

            # rope+bias rows first on the scalar ring (they gate every group's
            # final QK matmul), then the big K^T lora pair-blobs.
            kr = singles.tile([RR, RPC, CAP], kv_dt, tag="kr")
            nc.scalar.dma_start(out=kr, in_=ktr)
            klt = []
            for p in range(2):
                eng = nc.sync if p == 0 else nc.scalar
                kl = singles.tile([128, 2, 4, CAP], kv_dt, tag=f"kl{p}")
                eng.dma_start(out=kl, in_=ktl[p])
                klt.append(kl)

            # vh chunks: first half on the gpsimd SWDGE ring (lands during
            # pass A), the rest behind the K blobs on sync/scalar.
            vengs = [nc.gpsimd, nc.sync, nc.scalar]
            vts = []
            for ci, (t0, t1) in enumerate(VC):
                vt = singles.tile([BS, t1 - t0, RPC, KVL], kv_dt, tag=f"v{ci}")
                vengs[ci].dma_start(out=vt, in_=vh[:, t0:t1, :, :])
                vts.append((t0, t1, vt))

            ident = singles.tile([HP, HP], p_dt)
            make_identity(nc, ident)

            p_all = singles.tile([HP, T, BS], p_dt)
            sums = stats.tile([HP, T], f32)

            # ---- pass A: QK(+bias) -> exp -> p tiles + per-group sums ----
            for i, gt in enumerate(GT):
                Ni = gt * BS
                oi = OFFS[i] * BS
                pa = pap.tile([HP, 512], f32)
                for c in range(4):
                    for r in range(RPC):
                        nc.tensor.matmul(
                            pa[RST * r : RST * r + H, 0:Ni],
                            qt1[:, r, c, :],
                            klt[r // 2][:, r % 2, c, oi : oi + Ni],
                            start=(c == 0),
                            stop=False,
                            tile_position=(0, RST * r),
                        )
                for r in range(RPC):
                    nc.tensor.matmul(
                        pa[RST * r : RST * r + H, 0:Ni],
                        qt2[:, r, :],
                        kr[:, r, oi : oi + Ni],
                        start=False,
                        stop=True,
                        tile_position=(0, RST * r),
                    )
                bsl = slice(OFFS[i], OFFS[i] + gt)
                for j in range(gt):
                    idx = OFFS[i] + j
                    nc.scalar.activation(
                        out=p_all[:, idx, :],
                        in_=pa[:, BS * j : BS * (j + 1)],
                        func=mybir.ActivationFunctionType.Exp,
                        bias=0.0,
                        scale=1.0,
                    )
                nc.vector.reduce_sum(
                    out=sums[:, bsl], in_=p_all[:, bsl, :], axis=mybir.AxisListType.X
                )

            # ---- pass B: transpose p, PV accumulate (order-free over idx) ----
            po = pop.tile([HP, KVL], f32)
            for t0, t1, vt in vts:
                for idx in range(t0, t1):
                    ptp = ptpp.tile([BS, HP], p_dt)
                    nc.tensor.transpose(ptp, p_all[:, idx, :], ident)
                    pt_sb = pp.tile([BS, HP], kv_dt)
                    nc.vector.tensor_copy(pt_sb, ptp)
                    for r in range(RPC):
                        nc.tensor.matmul(
                            po[RST * r : RST * r + H, :],
                            pt_sb[:, RST * r : RST * r + H],
                            vt[:, idx - t0, r, :],
                            start=(idx == 0),
                            stop=(idx == T - 1),
                            tile_position=(0, RST * r),
                        )

            # ---- finalize: o = po / rowsum ----
            gs = stats.tile([HP, 1], f32)
            rgs = stats.tile([HP, 1], f32)
            nc.vector.reduce_sum(out=gs, in_=sums, axis=mybir.AxisListType.X)
            nc.vector.reciprocal(rgs, gs)
            o_sb = singles.tile([HP, KVL], f32)
            nc.vector.tensor_scalar_mul(o_sb, po, rgs[:, 0:1])
            for r in range(RPC):
                oeng = nc.sync if r % 2 == 0 else nc.scalar
                oeng.dma_start(out=o[r], in_=o_sb[RST * r : RST * r + H, :])

    nc.compile()
    return nc


def _get_nc(T):
    key = (T, KV_DT, P_DT)
    if key not in _NC_CACHE:
        _NC_CACHE[key] = _build(T, *key[1:])
    return _NC_CACHE[key]


def kernel(query, key_cache, block_mapping, block_bias, block_list, block_groups):
    global LAST_RESULTS
    query = np.asarray(query)
    key_cache = np.asarray(key_cache, dtype=np.float32)
    block_bias = np.asarray(block_bias, dtype=np.float32)
    block_list = np.asarray(block_list)
    block_groups = np.asarray(block_groups)

    # Sort blocks by request; each request must own exactly BPS blocks.
    perm = np.argsort(block_groups, kind="stable")
    bg = block_groups[perm]
    assert (np.bincount(bg, minlength=B) == BPS).all()
    bl = block_list[perm]
    bias = block_bias[perm]

    np_kv = _np_of(KV_DT)

    # Pack only used (bias > -1e8) positions per request.
    used = bias > -1.0e8                       # [NB, BS]
    per_req_used = used.reshape(B, BPS * BS).sum(1)
    T = max(1, int(-(-int(per_req_used.max()) // BS)))
    CAP = T * BS

    # Padded per-request K (d-major) / V (s-major) / bias, gathered once.
    kd = np.zeros((B, DR, CAP), np.float32)    # [b, 577, CAP]: K^T rows + bias row
    vv = np.zeros((B, CAP, KVL), np_kv)        # [b, CAP, 512]
    kd[:, D, :] = NEG
    for b in range(B):
        blocks = bl[BPS * b : BPS * (b + 1)]
        m = used[BPS * b : BPS * (b + 1)].reshape(-1)          # [2048]
        pages = key_cache[blocks].reshape(BPS * BS, D)          # [2048, 576]
        pos = np.nonzero(m)[0]
        L = pos.size
        sel = pages[pos]                                        # [L, 576]
        kd[b, :D, :L] = sel.T
        kd[b, D, :L] = bias[BPS * b : BPS * (b + 1)].reshape(-1)[pos]
        vv[b, :L, :] = sel[:, :KVL].astype(np_kv)

    kdb = kd.astype(np_kv)

    nc = _get_nc(T)
    in_maps = []
    for cc in range(NCORES):
        rs = slice(RPC * cc, RPC * (cc + 1))
        # lora rows -> [pair, 128, r2, chunk, CAP]
        ktl = np.ascontiguousarray(
            kdb[rs, : 4 * 128, :]
            .reshape(2, 2, 4, 128, CAP)
            .transpose(0, 3, 1, 2, 4)
        )
        ktr = np.ascontiguousarray(kdb[rs, 512:DR, :].transpose(1, 0, 2))
        # v tiles -> [s, tile, r, e]
        vhh = np.ascontiguousarray(
            vv[rs].reshape(RPC, T, BS, KVL).transpose(2, 1, 0, 3)
        )
        qtt = np.empty((RPC, DR, H), np_kv)
        qtt[:, :D, :] = (SCALE * query[rs]).transpose(0, 2, 1)
        qtt[:, D, :] = 1.0
        in_maps.append({"ktl": ktl, "ktr": ktr, "vh": vhh, "qt": qtt})

    res = run_bass_kernel_spmd(nc, in_maps, list(range(NCORES)), trace=TRACE)
    if TRACE:
        LAST_RESULTS = res
    return np.concatenate(
        [res.results[i]["o"] for i in range(NCORES)], axis=0
    ).astype(np.float32)


# revision 8
# speedup vs baseline: 1.2716x; 1.2716x over previous
"""MLA decode paged attention (flat_pa_mla latent-cache path) on 8 TRN2 NeuronCores.

Sharding: data-parallel over the batch axis — each core owns 4 complete requests
and computes its slice of the output independently, no collectives.

Optimizations over the dense baseline (the kernel is DMA-bound, so HW time
tracks HBM bytes):

1. Masked-position packing: block_bias masks the unused tail of every paged
   block (avg usage 64/128) and masked positions contribute exactly zero, so
   host prep gathers ONLY the used positions of each request's 16 blocks.

2. Ragged per-slot capacities: requests are sorted by used-position count and
   dealt so slot k on every core gets the (8k..8k+7)-ranked requests; slot k's
   tile count T[k] = ceil(max_used_in_slot/128). ~45% fewer bytes than dense.

3. No max-subtraction: logits are O(10) for this distribution, so exp() is
   safe in f32: p = exp(qk + bias), o = (sum p v) / (sum p). This removes the
   all-tiles max/rescale barrier between QK and PV — PV accumulation pipelines
   tile-by-tile inside the DMA stream.

4. DMA shape discipline: few large descriptors (>=8KB per-partition runs),
   <=17 dma_starts total (semaphore-lane reuse otherwise false-serializes
   issue), K blobs early (they gate pass A), V in ~6 chunks alternating rings
   so PV drains incrementally and the post-DMA tail is short.

Device (per core), 4 requests in lockstep at 32-partition stride (PE column
groups via tile_position):
  pass A per position-group (<=4 tiles): per request 5 PE matmuls accumulate
  qk+bias into a PSUM bank (lhsT = qt chunk, rhs = K^T blob slice); ACT exp ->
  p tiles (bf16), DVE per-group sums. Groups common to all slots run jointly
  on 128 partitions; ragged remainders run per-slot on 32-partition slices.
  pass B per tile: PE-transpose p, per-slot PV matmuls accumulate [128,512].
  Finalize: o = po * (1/sum p) broadcast, 4 small DMAs out.
"""

import numpy as np

import concourse.bass as bass
import concourse.mybir as mybir
import concourse.tile as tile
from concourse import bacc
from concourse.bass_utils import run_bass_kernel_spmd
from concourse.masks import make_identity

B = 32
H = 16
KVL = 512
ROPE = 64
D = KVL + ROPE          # 576
BS = 128
BPS = 16                # blocks per request (input format)
NB = B * BPS            # 512
SCALE = 192 ** -0.5
NEG = -1.0e9
NCORES = 8
RPC = B // NCORES       # 4 requests per core
DR = D + 1              # 577 rows: 576 latent+rope dims + 1 bias row
RR = DR - 512           # 65 rope+bias rows
RST = 32                # per-request partition stride (PE col groups are 32-wide)
HP = RPC * RST          # 128 partitions spanned by packed per-request ops
NVCH = 6                # vh DMA chunk count

KV_DT = mybir.dt.bfloat16
P_DT = mybir.dt.bfloat16

TRACE = False           # set True (with profhook installed) to NTFF-profile
LAST_RESULTS = None     # BassKernelResults of the last kernel() call when TRACE

_NC_CACHE = {}


def _np_of(dt):
    import ml_dtypes

    return {mybir.dt.float32: np.float32, mybir.dt.bfloat16: ml_dtypes.bfloat16}[dt]


def _plan(T):
    """Static schedule pieces derived from per-slot tile counts T (len RPC)."""
    ncommon = min(T) // 4                      # joint groups of 4 tiles
    rag = [(k, 4 * ncommon, t - 4 * ncommon) for k, t in enumerate(T)
           if t > 4 * ncommon]                 # (slot, tile0, ntiles)
    seq = [(idx, k) for idx in range(max(T)) for k in range(RPC) if idx < T[k]]
    # vh chunks: split seq into NVCH nearly-equal contiguous runs
    nt = len(seq)
    base, extra = divmod(nt, NVCH)
    cuts = [0]
    for c in range(NVCH):
        cuts.append(cuts[-1] + base + (1 if c < extra else 0))
    chunks = [(cuts[i], cuts[i + 1]) for i in range(NVCH) if cuts[i] < cuts[i + 1]]
    koffs = np.cumsum([0] + [t * BS for t in T]).tolist()  # kr col offsets
    return ncommon, rag, seq, chunks, koffs


def _build(T, kv_dt, p_dt):
    T = list(T)
    f32 = mybir.dt.float32
    ncommon, rag, seq, chunks, koffs = _plan(T)
    NT = len(seq)
    TCAP = koffs[-1]
    nc = bacc.Bacc("TRN2", target_bir_lowering=False, debug=False)
    ktl = [
        nc.dram_tensor(f"ktl{k}", [128, 4, T[k] * BS], kv_dt, kind="ExternalInput").ap()
        for k in range(RPC)
    ]
    ktr = nc.dram_tensor("ktr", [RR, TCAP], kv_dt, kind="ExternalInput").ap()
    vh = nc.dram_tensor("vh", [BS, NT, KVL], kv_dt, kind="ExternalInput").ap()
    qta = nc.dram_tensor("qta", [128, RPC, 4, H], kv_dt, kind="ExternalInput").ap()
    qtb = nc.dram_tensor("qtb", [RR, RPC, H], kv_dt, kind="ExternalInput").ap()
    o = nc.dram_tensor("o", [RPC, H, KVL], f32, kind="ExternalOutput").ap()

    with tile.TileContext(nc) as tc:
        with (
            tc.tile_pool(name="singles", bufs=1) as singles,
            tc.tile_pool(name="pp", bufs=4) as pp,
            tc.tile_pool(name="stats", bufs=4) as stats,
            tc.tile_pool(name="pap", bufs=2, space="PSUM") as pap,
            tc.tile_pool(name="ptpp", bufs=2, space="PSUM") as ptpp,
            tc.tile_pool(name="pop", bufs=1, space="PSUM") as pop,
        ):
            # qt first (lhsT of every pass-A matmul), pre-swizzled on host.
            qt1 = singles.tile([128, RPC, 4, H], kv_dt)
            nc.gpsimd.dma_start(out=qt1, in_=qta)
            qt2 = singles.tile([RR, RPC, H], kv_dt)
            nc.gpsimd.dma_start(out=qt2, in_=qtb)

            # K blobs first (they gate pass A): rope+bias rows lead the scalar
            # ring; lora blobs alternate rings biggest-first.
            kr = singles.tile([RR, TCAP], kv_dt, tag="kr")
            nc.scalar.dma_start(out=kr, in_=ktr)
            klt = []
            for k in range(RPC):
                eng = nc.sync if k % 2 == 0 else nc.scalar
                kl = singles.tile([128, 4, T[k] * BS], kv_dt, tag=f"kl{k}")
                eng.dma_start(out=kl, in_=ktl[k])
                klt.append(kl)

            # vh chunks alternate rings; sync leads (scalar carries kr extra).
            vts = []
            for ci, (g0, g1) in enumerate(chunks):
                vt = singles.tile([BS, g1 - g0, KVL], kv_dt, tag=f"v{ci}")
                veng = nc.sync if ci % 2 == 0 else nc.scalar
                veng.dma_start(out=vt, in_=vh[:, g0:g1, :])
                vts.append(vt)

            ident = singles.tile([HP, HP], p_dt)
            make_identity(nc, ident)

            T0 = max(T)
            p_all = singles.tile([HP, T0, BS], p_dt)
            sums = stats.tile([HP, T0], f32)
            nc.vector.memset(sums, 0.0)

            # ---- pass A: QK(+bias) -> exp -> p tiles + per-group sums ----
            def qk_group(k, oi, Ni, pa):
                for c in range(4):
                    nc.tensor.matmul(
                        pa[RST * k : RST * k + H, 0:Ni],
                        qt1[:, k, c, :],
                        klt[k][:, c, oi : oi + Ni],
                        start=(c == 0),
                        stop=False,
                        tile_position=(0, RST * k),
                    )
                nc.tensor.matmul(
                    pa[RST * k : RST * k + H, 0:Ni],
                    qt2[:, k, :],
                    kr[:, koffs[k] + oi : koffs[k] + oi + Ni],
                    start=False,
                    stop=True,
                    tile_position=(0, RST * k),
                )

            for i in range(ncommon):
                oi = 4 * i * BS
                pa = pap.tile([HP, 512], f32)
                for k in range(RPC):
                    qk_group(k, oi, 512, pa)
                for j in range(4):
                    idx = 4 * i + j
                    nc.scalar.activation(
                        out=p_all[:, idx, :],
                        in_=pa[:, BS * j : BS * (j + 1)],
                        func=mybir.ActivationFunctionType.Exp,
                        bias=0.0,
                        scale=1.0,
                    )
                nc.vector.reduce_sum(
                    out=sums[:, 4 * i : 4 * i + 4],
                    in_=p_all[:, 4 * i : 4 * i + 4, :],
                    axis=mybir.AxisListType.X,
                )

            if rag:
                pa = pap.tile([HP, 512], f32)
                for k, t0, nt_k in rag:
                    qk_group(k, t0 * BS, nt_k * BS, pa)
                for k, t0, nt_k in rag:
                    rsl = slice(RST * k, RST * k + RST)
                    for j in range(nt_k):
                        nc.scalar.activation(
                            out=p_all[rsl, t0 + j, :],
                            in_=pa[rsl, BS * j : BS * (j + 1)],
                            func=mybir.ActivationFunctionType.Exp,
                            bias=0.0,
                            scale=1.0,
                        )
                    nc.vector.reduce_sum(
                        out=sums[rsl, t0 : t0 + nt_k],
                        in_=p_all[rsl, t0 : t0 + nt_k, :],
                        axis=mybir.AxisListType.X,
                    )

            # ---- pass B: transpose p per tile, PV accumulate ----
            po = pop.tile([HP, KVL], f32)
            ptcache = {}
            first = {k: True for k in range(RPC)}
            last_g = {}
            for g, (idx, k) in enumerate(seq):
                last_g[k] = g
            ci = 0
            for g, (idx, k) in enumerate(seq):
                while g >= chunks[ci][1]:
                    ci += 1
                if idx not in ptcache:
                    ptp = ptpp.tile([BS, HP], p_dt, tag="ptp")
                    nc.tensor.transpose(ptp, p_all[:, idx, :], ident)
                    pt_sb = pp.tile([BS, HP], kv_dt, tag="pt")
                    nc.vector.tensor_copy(pt_sb, ptp)
                    ptcache[idx] = pt_sb
                pt_sb = ptcache[idx]
                nc.tensor.matmul(
                    po[RST * k : RST * k + H, :],
                    pt_sb[:, RST * k : RST * k + H],
                    vts[ci][:, g - chunks[ci][0], :],
                    start=first[k],
                    stop=(g == last_g[k]),
                    tile_position=(0, RST * k),
                )
                first[k] = False

            # ---- finalize: o = po / rowsum ----
            gs = stats.tile([HP, 1], f32)
            rgs = stats.tile([HP, 1], f32)
            nc.vector.reduce_sum(out=gs, in_=sums, axis=mybir.AxisListType.X)
            nc.vector.reciprocal(rgs, gs)
            o_sb = singles.tile([HP, KVL], f32)
            nc.vector.tensor_scalar_mul(o_sb, po, rgs[:, 0:1])
            for r in range(RPC):
                oeng = nc.sync if r % 2 == 0 else nc.scalar
                oeng.dma_start(out=o[r], in_=o_sb[RST * r : RST * r + H, :])

    nc.compile()
    return nc


def _get_nc(T):
    key = (tuple(T), KV_DT, P_DT)
    if key not in _NC_CACHE:
        _NC_CACHE[key] = _build(key[0], KV_DT, P_DT)
    return _NC_CACHE[key]


def kernel(query, key_cache, block_mapping, block_bias, block_list, block_groups):
    global LAST_RESULTS
    query = np.asarray(query)
    key_cache = np.asarray(key_cache, dtype=np.float32)
    block_bias = np.asarray(block_bias, dtype=np.float32)
    block_list = np.asarray(block_list)
    block_groups = np.asarray(block_groups)

    # Sort blocks by request; each request must own exactly BPS blocks.
    perm = np.argsort(block_groups, kind="stable")
    bg = block_groups[perm]
    assert (np.bincount(bg, minlength=B) == BPS).all()
    bl = block_list[perm]
    bias = block_bias[perm]

    np_kv = _np_of(KV_DT)

    # Pack only used (bias > -1e8) positions; sort requests by length and deal
    # round-robin: slot k on core c gets rank 8k+c.
    used = bias > -1.0e8                       # [NB, BS]
    per_req_used = used.reshape(B, BPS * BS).sum(1)
    order = np.argsort(-per_req_used, kind="stable")
    T = []
    for k in range(RPC):
        mx = int(per_req_used[order[k * NCORES : (k + 1) * NCORES]].max())
        T.append(max(1, -(-mx // BS)))

    ncommon, rag, seq, chunks, koffs = _plan(T)
    NT = len(seq)
    TCAP = koffs[-1]

    # Gather per-request packed K^T (d-major, with bias row) and V (s-major).
    caps = {b: T[k] * BS for k in range(RPC) for b in order[k * NCORES : (k + 1) * NCORES]}
    kd = {}
    vv = {}
    for b in range(B):
        cap = caps[b]
        blocks = bl[BPS * b : BPS * (b + 1)]
        m = used[BPS * b : BPS * (b + 1)].reshape(-1)
        pages = key_cache[blocks].reshape(BPS * BS, D)
        pos = np.nonzero(m)[0]
        L = pos.size
        sel = pages[pos]
        kb = np.zeros((DR, cap), np.float32)
        kb[D, :] = NEG
        kb[:D, :L] = sel.T
        kb[D, :L] = bias[BPS * b : BPS * (b + 1)].reshape(-1)[pos]
        kd[b] = kb.astype(np_kv)
        vb = np.zeros((cap, KVL), np_kv)
        vb[:L] = sel[:, :KVL].astype(np_kv)
        vv[b] = vb

    nc = _get_nc(T)
    in_maps = []
    for cc in range(NCORES):
        reqs = [order[k * NCORES + cc] for k in range(RPC)]
        im = {}
        for k in range(RPC):
            kb = kd[reqs[k]]
            im[f"ktl{k}"] = np.ascontiguousarray(
                kb[: 4 * 128].reshape(4, 128, T[k] * BS).transpose(1, 0, 2)
            )
        im["ktr"] = np.concatenate([kd[reqs[k]][512:DR] for k in range(RPC)], axis=1)
        vts = np.empty((BS, NT, KVL), np_kv)
        for g, (idx, k) in enumerate(seq):
            vts[:, g, :] = vv[reqs[k]][idx * BS : (idx + 1) * BS]
        im["vh"] = vts
        qtt = np.empty((RPC, DR, H), np_kv)
        qtt[:, :D, :] = (SCALE * query[reqs]).transpose(0, 2, 1)
        qtt[:, D, :] = 1.0
        im["qta"] = np.ascontiguousarray(
            qtt[:, : 4 * 128, :].reshape(RPC, 4, 128, H).transpose(2, 0, 1, 3)
        )
        im["qtb"] = np.ascontiguousarray(qtt[:, 512:DR, :].transpose(1, 0, 2))
        in_maps.append(im)

    res = run_bass_kernel_spmd(nc, in_maps, list(range(NCORES)), trace=TRACE)
    if TRACE:
        LAST_RESULTS = res

    out = np.empty((B, H, KVL), np.float32)
    for cc in range(NCORES):
        oc = res.results[cc]["o"]
        for k in range(RPC):
            out[order[k * NCORES + cc]] = oc[k]
    return out
